# revision 12
# baseline (speedup 1.0000x reference)
"""CEHessianCalculator diagonal-Hessian kernel for 8 Trainium2 NeuronCores.

Math (reference):
    val     = x @ W.T + b                     [B, C]
    softmax = exp(val) / rowsum(exp(val))     [B, C]
    out     = mean_b(softmax @ W^2 - (softmax @ W)^2)   [D]

Device algorithm (C-sharded over 8 cores, b-chunked):
  Per core, with a local C-slice (C_LOC rows of W, padded):
    eb   = exp(b_local)                           (folds the bias: exp(v+b) = exp(v)*eb)
    WtT  = W_local.T            [D, C_LOC]        (PE transposes, resident in SBUF)
    W'   = W_local * eb[:,None] [C_LOC, D]        (resident)
    W''  = W_local^2 * eb[:,None]                 (resident)
    for each 512-row b-chunk:
       v    = WtT-tile matmuls -> logits.T [c, b] (PSUM, two tiles per slot)
       ev   = exp(v)                              (ACT)
       U   += W'.T @ ev   (PSUM accumulate)
       Q   += W''.T @ ev  (PSUM accumulate)
       s   += eb-weighted column-accumulation of ev (DVE fused mul-add)
    U, Q transposed to [b, d] and packed, with s, into one DRAM buffer laid
    out so a single ReduceScatter(add) hands core k the full-C U/Q/s of its
    own b-chunk; it finishes mean_b(Q/s - (U/s)^2) locally -> [D] partials;
    the host adds the 8 partials.

float32r matmuls (11-bit-mantissa operands, fp32 accumulate) run at full
1 cycle/row PE speed; per-element rounding errors average out over C=50K.
Emission is software-pipelined (pair p's logits+exp issued one pair ahead
of its U/Q/s consumers) so PE, ACT and DVE overlap with 3 psv slots.
"""

import numpy as np
from contextlib import ExitStack

import concourse.bass as bass
import concourse.bacc as bacc
import concourse.tile as tile
from concourse import mybir
from concourse.bass_utils import run_bass_kernel_spmd
from concourse.masks import make_identity

F32 = mybir.dt.float32
AFT = mybir.ActivationFunctionType
ALU = mybir.AluOpType

B, C, D = 4096, 50257, 128
NCORE = 8
T = 50                      # W tiles (of 128 rows) per core
C_LOC = T * 128             # 6400
C_PAD = NCORE * C_LOC       # 51200
NCHUNK = 8
CH = 512                    # b rows per chunk
B_PAD_VAL = -40.0           # exp(-40) ~ 4e-18: padded classes contribute nothing
MM_DT = mybir.dt.float32r
SROW = 128 + 128 + 1        # per-chunk rows in the fused collective buffer


def _build():
    nc = bacc.Bacc("TRN2", target_bir_lowering=False, debug=False, num_devices=NCORE)
    x_d = nc.dram_tensor("x", [B, D], F32, kind="ExternalInput").ap()
    W_d = nc.dram_tensor("Wl", [C_LOC, D], F32, kind="ExternalInput").ap()
    b_d = nc.dram_tensor("bl", [C_LOC], F32, kind="ExternalInput").ap()
    out_d = nc.dram_tensor("out", [D], F32, kind="ExternalOutput").ap()

    with tile.TileContext(nc) as tc, ExitStack() as ctx:
        const = ctx.enter_context(tc.tile_pool(name="const", bufs=1))
        wres = ctx.enter_context(tc.tile_pool(name="wres", bufs=1))
        wld = ctx.enter_context(tc.tile_pool(name="wld", bufs=3))
        sb = ctx.enter_context(tc.tile_pool(name="sb", bufs=3))
        evp = ctx.enter_context(tc.tile_pool(name="evp", bufs=8))
        fin = ctx.enter_context(tc.tile_pool(name="fin", bufs=1))
        pv = ctx.enter_context(tc.tile_pool(name="pv", bufs=2, space="PSUM"))
        pacc = ctx.enter_context(tc.tile_pool(name="pacc", bufs=1, space="PSUM"))
        pprep = ctx.enter_context(tc.tile_pool(name="pprep", bufs=1, space="PSUM"))
        dram = ctx.enter_context(tc.tile_pool(name="dram", bufs=1, space="DRAM"))

        ident = const.tile([128, 128], F32)
        make_identity(nc, ident[:])
        ones1 = const.tile([1, 128], F32)
        nc.gpsimd.memset(ones1[:], 1.0)

        b_sb = const.tile([128, T], F32)
        nc.sync.dma_start(b_sb[:], b_d.rearrange("(t c) -> c t", c=128))
        eb = const.tile([128, T], F32)
        nc.scalar.activation(eb[:], b_sb[:], AFT.Exp)
        ebr_t = const.tile([128, T], MM_DT)
        nc.vector.tensor_copy(ebr_t[:], eb[:])

        WtT = wres.tile([128, C_LOC], MM_DT)   # [d, c_loc]
        Wp = wres.tile([128, C_LOC], MM_DT)    # [c(tile-part), d] per 128-col block
        W2p = wres.tile([128, C_LOC], MM_DT)
        xT = wres.tile([128, B], MM_DT)        # [d, b]

        # ---- prep: xT = x.T (PE transpose, 4 tiles per PSUM bank) ----
        for g in range(B // 512):
            pst = (pprep if g % 2 else pv).tile([128, 512], F32, tag="v")
            xb = wld.tile([128, 512], F32, tag="xload")
            nc.sync.dma_start(
                xb[:].rearrange("p (j d) -> p j d", d=128),
                x_d[g * 512:(g + 1) * 512, :].rearrange("(j p) d -> p j d", p=128))
            for j in range(4):
                nc.tensor.transpose(pst[:, j * 128:(j + 1) * 128],
                                    xb[:, j * 128:(j + 1) * 128], ident[:])
            nc.scalar.activation(xT[:, g * 512:(g + 1) * 512], pst[:], AFT.Copy)

        # ---- prep: W residents ----
        n_wg = (T + 3) // 4
        for g in range(n_wg):
            tg = min(4, T - g * 4)
            pst = (pprep if g % 2 else pv).tile([128, 512], F32, tag="v")
            wg_sb = wld.tile([128, 512], F32, tag="wload")
            nc.sync.dma_start(
                wg_sb[:, :tg * 128].rearrange("p (j d) -> p j d", d=128),
                W_d[g * 512:g * 512 + tg * 128, :].rearrange(
                    "(j p) d -> p j d", p=128))
            for j in range(tg):
                t = g * 4 + j
                wt = wg_sb[:, j * 128:(j + 1) * 128]
                nc.tensor.transpose(pst[:, j * 128:(j + 1) * 128], wt, ident[:])
                ebt = eb[:, t:t + 1]
                nc.vector.tensor_scalar_mul(
                    Wp[:, t * 128:(t + 1) * 128], wt, ebt)
                nc.vector.scalar_tensor_tensor(
                    W2p[:, t * 128:(t + 1) * 128], wt, ebt, wt,
                    op0=ALU.mult, op1=ALU.mult)
            nc.scalar.activation(
                WtT[:, g * 512:g * 512 + tg * 128], pst[:, :tg * 128], AFT.Copy)

        # ---- main: b-chunks ----
        # fused collective layout: chunk h owns rows [h*SROW, (h+1)*SROW):
        # U [128 d-rows x 512 b] | Q [128 x 512] | s (1 row of 512)
        S_dram = dram.tile([NCHUNK * SROW, CH], F32, tag="Sd")
        s_all = fin.tile([128, 4 * NCHUNK], F32, tag="sall")
        NP = T // 2

        for h in range(NCHUNK):
            U_ps = pacc.tile([128, CH], F32, tag="U")
            Q_ps = pacc.tile([128, CH], F32, tag="Q")
            s_ps = pacc.tile([1, CH], F32, tag="s")
            s_acc = sb.tile([128, CH], F32, tag="sacc")
            xs = xT[:, h * CH:(h + 1) * CH]
            evs = {}
            ebf = eb[:]
            ebr = ebr_t[:]
            # software-pipelined emission: pair p's logits+exp are issued one
            # pair ahead of its U/Q/s consumers, so PE keeps psv-slot work in
            # flight while ACT runs exp
            for p in range(NP + 1):
                if p < NP:
                    t0, t1 = 2 * p, 2 * p + 1
                    psv = pv.tile([128, 2 * CH], F32, tag="v")
                    nc.tensor.matmul(psv[:, 0:CH],
                                     WtT[:, t0 * 128:(t0 + 1) * 128],
                                     xs, start=True, stop=True)
                    nc.tensor.matmul(psv[:, CH:2 * CH],
                                     WtT[:, t1 * 128:(t1 + 1) * 128],
                                     xs, start=True, stop=True)
                    ev = evp.tile([128, 2 * CH], MM_DT, tag="ev")
                    nc.scalar.activation(ev[:], psv[:], AFT.Exp)
                    evs[p] = ev
                if p == 0:
                    continue
                q = p - 1
                t0, t1 = 2 * q, 2 * q + 1
                ev = evs.pop(q)
                ev0 = ev[:, 0:CH]
                ev1 = ev[:, CH:2 * CH]
                nc.tensor.matmul(U_ps[:], Wp[:, t0 * 128:(t0 + 1) * 128], ev0,
                                 start=(q == 0), stop=False)
                nc.tensor.matmul(U_ps[:], Wp[:, t1 * 128:(t1 + 1) * 128], ev1,
                                 start=False, stop=(q == NP - 1))
                nc.tensor.matmul(Q_ps[:], W2p[:, t0 * 128:(t0 + 1) * 128], ev0,
                                 start=(q == 0), stop=False)
                nc.tensor.matmul(Q_ps[:], W2p[:, t1 * 128:(t1 + 1) * 128], ev1,
                                 start=False, stop=(q == NP - 1))
                # s: one half-pair on PE (psum-accumulated M=1 matmul), the
                # other on DVE -- balances the two engines
                tp, tv = (t0, t1) if q % 2 == 0 else (t1, t0)
                evp_, evv = (ev0, ev1) if q % 2 == 0 else (ev1, ev0)
                nc.tensor.matmul(s_ps[:], ebr[:, tp:tp + 1], evp_,
                                 start=(q == 0), stop=(q == NP - 1))
                evvf = evv.bitcast(F32)
                if q == 0:
                    nc.vector.tensor_scalar_mul(s_acc[:], evvf, ebf[:, tv:tv + 1])
                else:
                    nc.vector.scalar_tensor_tensor(
                        s_acc[:], evvf, ebf[:, tv:tv + 1], s_acc[:],
                        op0=ALU.mult, op1=ALU.add)

            nc.vector.tensor_add(s_acc[0:1, :], s_acc[0:1, :], s_ps[0:1, :])
            # s: transpose c->free then reduce along free dim (keeps all DMAs
            # multi-partition; single-partition DMAs fail NEFF load)
            pss = pv.tile([128, CH], F32, tag="v")
            for j in range(4):
                nc.tensor.transpose(pss[:, j * 128:(j + 1) * 128],
                                    s_acc[:, j * 128:(j + 1) * 128], ident[:])
            for j in range(4):
                nc.vector.tensor_reduce(
                    s_all[:, h * 4 + j:h * 4 + j + 1],
                    pss[:, j * 128:(j + 1) * 128],
                    axis=mybir.AxisListType.X, op=ALU.add)

            # U/Q: PSUM -> SBUF, export untransposed ([d, b] layout)
            for acc_ps, roff in ((U_ps, 0), (Q_ps, 128)):
                a_sb = sb.tile([128, CH], F32, tag="acc_sb")
                nc.scalar.activation(a_sb[:], acc_ps[:], AFT.Copy)
                r0 = h * SROW + roff
                nc.sync.dma_start(S_dram[r0:r0 + 128, :], a_sb[:])

        for h in range(NCHUNK):
            r0 = h * SROW + 256
            nc.sync.dma_start(
                S_dram[r0:r0 + 1, :].rearrange("one (j p) -> p (one j)", p=128),
                s_all[:, h * 4:(h + 1) * 4])

        # ---- one fused reduce-scatter; each core finishes its own b-chunk ----
        S_rs = dram.tile([SROW, CH], F32, tag="Srs")
        nc.gpsimd.collective_compute(
            "ReduceScatter", ALU.add, replica_groups=[list(range(NCORE))],
            ins=[S_dram[:]], outs=[S_rs[:]])

        Urs_sb = fin.tile([128, CH], F32, tag="Ursb")
        nc.sync.dma_start(Urs_sb[:], S_rs[0:128, :])
        Qrs_sb = fin.tile([128, CH], F32, tag="Qrsb")
        nc.sync.dma_start(Qrs_sb[:], S_rs[128:256, :])
        srs_sb = fin.tile([128, 4], F32, tag="srsb")
        nc.sync.dma_start(srs_sb[:],
                          S_rs[256:257, :].rearrange("one (j p) -> p (one j)", p=128))
        r_sb = fin.tile([128, 4], F32, tag="rsb")
        nc.vector.reciprocal(r_sb[:], srs_sb[:])
        # r columns -> partition-0 rows via PE transposes, then broadcast
        # to [128, 512] with K=1 ones-matmuls
        rT_ps = pprep.tile([128, 512], F32, tag="v")
        for j in range(4):
            nc.tensor.transpose(rT_ps[0:1, j * 128:(j + 1) * 128],
                                r_sb[:, j:j + 1], ident[:])
        r4 = fin.tile([1, CH], F32, tag="r4")
        nc.vector.tensor_copy(r4[:], rT_ps[0:1, :])
        rb_ps = pacc.tile([128, CH], F32, tag="U")
        for j in range(4):
            nc.tensor.matmul(rb_ps[:, j * 128:(j + 1) * 128], ones1[:],
                             r4[0:1, j * 128:(j + 1) * 128], start=True, stop=True)
        r_bc = fin.tile([128, CH], F32, tag="rbc")
        nc.vector.tensor_copy(r_bc[:], rb_ps[:])

        t1 = fin.tile([128, CH], F32, tag="t1")
        nc.vector.tensor_mul(t1[:], Urs_sb[:], r_bc[:])     # U/s
        t2 = fin.tile([128, CH], F32, tag="t2")
        nc.vector.tensor_mul(t2[:], t1[:], t1[:])           # (U/s)^2
        t3 = fin.tile([128, CH], F32, tag="t3")
        nc.vector.tensor_mul(t3[:], Qrs_sb[:], r_bc[:])     # Q/s
        e_sb = fin.tile([128, CH], F32, tag="e")
        nc.vector.tensor_sub(e_sb[:], t3[:], t2[:])
        res_acc = fin.tile([128, 1], F32, tag="resacc")
        nc.vector.tensor_reduce(res_acc[:], e_sb[:],
                                axis=mybir.AxisListType.X, op=ALU.add)
        res_sb = fin.tile([128, 1], F32, tag="res_sb")
        nc.scalar.activation(res_sb[:], res_acc[:], AFT.Copy, scale=1.0 / B)
        nc.sync.dma_start(out_d[:].rearrange("(p one) -> p one", one=1), res_sb[:])

    nc.compile()
    return nc


_NC = None


def _get_nc():
    global _NC
    if _NC is None:
        _NC = _build()
    return _NC


def kernel(x, W, b, _trace=False, _trace_kwargs=None):
    x = np.ascontiguousarray(np.asarray(x, dtype=np.float32))
    W = np.asarray(W, dtype=np.float32)
    b = np.asarray(b, dtype=np.float32)
    assert x.shape == (B, D) and W.shape == (C, D) and b.shape == (C,)

    W_pad = np.zeros((C_PAD, D), dtype=np.float32)
    W_pad[:C] = W
    b_pad = np.full((C_PAD,), B_PAD_VAL, dtype=np.float32)
    b_pad[:C] = b

    in_maps = []
    for k in range(NCORE):
        in_maps.append({
            "x": x,
            "Wl": np.ascontiguousarray(W_pad[k * C_LOC:(k + 1) * C_LOC]),
            "bl": np.ascontiguousarray(b_pad[k * C_LOC:(k + 1) * C_LOC]),
        })

    nc = _get_nc()
    r = run_bass_kernel_spmd(
        nc, in_maps, list(range(NCORE)),
        trace=_trace, **(_trace_kwargs or {}))
    out = np.zeros((D,), dtype=np.float64)
    for k in range(NCORE):
        out += r.results[k]["out"].astype(np.float64)
    if _trace:
        return out.astype(np.float32), r
    return out.astype(np.float32)


if __name__ == "__main__":
    rng = np.random.default_rng(0)
    x = rng.standard_normal((B, D)).astype(np.float32)
    W = (0.01 * rng.standard_normal((C, D))).astype(np.float32)
    b = (0.01 * rng.standard_normal((C,))).astype(np.float32)
    got = kernel(x, W, b)
    val = x.astype(np.float64) @ W.astype(np.float64).T + b.astype(np.float64)
    e = np.exp(val)
    sm = e / e.sum(1, keepdims=True)
    ref = (sm @ (W.astype(np.float64) ** 2) - (sm @ W.astype(np.float64)) ** 2).mean(0)
    rel = np.abs(got - ref) / (np.abs(ref).max())
    print("scale-rel max err:", rel.max())
